# revision 13
# baseline (speedup 1.0000x reference)
"""Trainium2 Bass kernel for nn_MultiHeadAttention_21251498181338.

Music-Transformer-style MHA with relative position embeddings (Huang et al.
skew trick), B=2, L=2048, D=1024, H=16, causal mask.

Sharding: 8 cores = 2 batches x 4-head groups (tensor parallel per head).
Each core computes q/k/v projections for its 4 heads, causal attention with
relative position logits, and a partial output projection (Wo row-split).
Partials are summed on the host during unshard.

Device-side structure (per core):
  - Projections produce qh^T/kh^T in [head-depth on partitions] layout and
    vh in [keys on partitions] layout, so no transposes are needed anywhere
    except for the attention probabilities themselves.
  - P = exp(QK^T/8) * exp(Srel/8): the additive logit split is computed
    multiplicatively so the relative-position term can be skew-aligned
    independently of QK^T.
  - The skew is a single SBUF->SBUF DMA per (head, q-tile) using a flat
    access pattern with partition step (row_len - 1): row i is read with a
    column offset of -i, which is exactly the Huang et al. pad/reshape
    trick. Columns beyond the valid relative-index range are zeroed, which
    also implements the causal mask for free (P = Pqk * 0 = 0 there).
  - PV uses TensorE transposes of P tiles. The softmax denominators come
    for free from the fused multiply+reduce (tensor_tensor_reduce): the
    per-query row sums accumulate on the vector engine during the
    P = exp(QK)*exp(Srel) multiply, so no denominator matmuls are needed.
    The per-query reciprocals are turned into a partition-replicated
    [128, 128] tile via a DVE 32x32 stream-transpose plus two tiny
    SBUF->SBUF broadcast DMAs.
  - The two heads of each pair interleave their K=64 matmuls (different PE
    row-groups run concurrently) and share [128, P] psum tiles for PV via
    tile_position column halves, so both heads normalize in one op and
    land directly in the packed outT layout.
  - The attention output appears transposed [depth, queries], which is
    exactly the stationary-operand layout the output projection needs.
"""

import os
import sys

sys.path.insert(0, "/opt/trn_rl_repo")

import numpy as np
import ml_dtypes

import concourse.bass as bass
import concourse.mybir as mybir
import concourse.tile as tile
from concourse import bacc
from concourse.bass_utils import run_bass_kernel_spmd
from concourse.masks import make_identity

BF16 = mybir.dt.bfloat16
F32 = mybir.dt.float32
NPBF16 = ml_dtypes.bfloat16

B, L, DM, H, D = 2, 2048, 1024, 16, 64
HG = 4            # heads per core (head group)
NCORES = 8
P = 128
KT = DM // P      # 8 contraction tiles for projections
NIT = L // P      # 16 query tiles
SCALE = 1.0 / np.sqrt(D)  # 0.125

LAST_EXEC_NS = None

_PROG = None


def _ncj(it):
    # number of 512-wide key chunks for query tile `it` (causal)
    return it // 4 + 1


def build_program():
    nc = bacc.Bacc(
        "TRN2",
        target_bir_lowering=False,
        debug=False,
        enable_asserts=False,
        num_devices=NCORES,
    )

    # ---- External I/O ----
    xq = nc.dram_tensor("xq", [DM, L], BF16, kind="ExternalInput")  # q[b].T
    xk = nc.dram_tensor("xk", [DM, L], BF16, kind="ExternalInput")
    xv = nc.dram_tensor("xv", [DM, L], BF16, kind="ExternalInput")
    wq = nc.dram_tensor("wq", [DM, 2 * P], BF16, kind="ExternalInput")  # group cols
    wk = nc.dram_tensor("wk", [DM, 2 * P], BF16, kind="ExternalInput")
    wv = nc.dram_tensor("wv", [DM, 2 * P], BF16, kind="ExternalInput")
    wo = nc.dram_tensor("wo", [2, P, DM], BF16, kind="ExternalInput")  # [hp, 2h*64, dm]
    eT = nc.dram_tensor("eT", [2, P, L], BF16, kind="ExternalInput")   # [hp, 2h*64, r]
    bqk = nc.dram_tensor("bqk", [P, 4], F32, kind="ExternalInput")  # cols 0:2 bq, 2:4 bk
    mask2_d = nc.dram_tensor("mask2", [2, P], F32, kind="ExternalInput")
    bv_t = nc.dram_tensor("bv", [P, 2 * P], F32, kind="ExternalInput")  # row-replicated
    bo_t = nc.dram_tensor("bo", [P, DM], F32, kind="ExternalInput")     # row-replicated
    out = nc.dram_tensor("out", [L, DM], F32, kind="ExternalOutput")

    with tile.TileContext(nc) as tc:
        with (
            tc.tile_pool(name="persist", bufs=1) as pp,
            tc.tile_pool(name="work", bufs=2) as wp,
            tc.tile_pool(name="small", bufs=4) as sp,
        ):
            # ---- persistent SBUF tensors ----
            ident = pp.tile([P, P], BF16)
            make_identity(nc, ident)

            wq_sb = pp.tile([P, KT, 2 * P], BF16)
            nc.sync.dma_start(wq_sb, wq.ap().rearrange("(t p) c -> p t c", p=P))
            wk_sb = pp.tile([P, KT, 2 * P], BF16)
            nc.sync.dma_start(wk_sb, wk.ap().rearrange("(t p) c -> p t c", p=P))
            wv_sb = pp.tile([P, KT, 2 * P], BF16)
            nc.sync.dma_start(wv_sb, wv.ap().rearrange("(t p) c -> p t c", p=P))
            wo_sb = pp.tile([P, 2, DM], BF16)
            nc.sync.dma_start(wo_sb, wo.ap().rearrange("h p m -> p h m"))
            eT_sb = pp.tile([P, 2, L], BF16)
            nc.sync.dma_start(eT_sb, eT.ap().rearrange("h p r -> p h r"))
            bqk_sb = pp.tile([P, 4], F32)
            nc.sync.dma_start(bqk_sb, bqk.ap())
            bv_sb = pp.tile([P, 2 * P], F32)
            nc.sync.dma_start(bv_sb, bv_t.ap())
            bo_sb = pp.tile([P, DM], F32)
            nc.sync.dma_start(bo_sb, bo_t.ap())

            qhT = pp.tile([P, 2, L], BF16)   # [64*hl+d, hp, i]
            khT = pp.tile([P, 2, L], BF16)
            vh = pp.tile([P, NIT, HG, 66], BF16)  # [j in tile, jt, local head, d|1|pad]
            outT = pp.tile([P, 2, L], BF16)  # [64*hl+d, hp, i]

            # persistent exp(Srel) band buffers, zeroed ONCE: each use writes
            # exp values into [0:W] and the skew DMA reads zeros beyond W.
            # W grows monotonically with it, so the zero tail stays clean.
            XSE_W = 2176  # 2048 max band + 128 skew overhang
            xse_pp = [[pp.tile([P, XSE_W], BF16, name=f"xse{hl}_{par}")
                       for par in range(2)] for hl in range(2)]
            for hl in range(2):
                for par in range(2):
                    nc.gpsimd.memset(xse_pp[hl][par], 0.0)
            # fp32 identity for the reciprocal transpose-matmul, and the
            # head-half selector used to replicate each head's reciprocal
            # row across its 64 psum partitions.
            ident_f32 = pp.tile([P, P], F32)
            make_identity(nc, ident_f32)
            mask2 = pp.tile([2, P], F32)
            nc.sync.dma_start(mask2, mask2_d.ap())

            # ---- Stage 1: projections ----
            with (
                tc.tile_pool(name="xin", bufs=2) as xp,
                tc.tile_pool(name="ps1", bufs=4, space="PSUM") as ps1,
            ):
                # v projection first (vh used last; q/k streams lead into
                # stage 2 with the PE warm)
                xt = xp.tile([P, KT, L], BF16, tag="xin")
                for kt in range(KT):
                    nc.sync.dma_start(xt[:, kt, :],
                                      xv.ap()[kt * P:(kt + 1) * P, :])
                for jt in range(NIT):
                    ps = ps1.tile([P, 2 * P], F32, tag="psv")
                    for kt in range(KT):
                        nc.tensor.matmul(
                            ps,
                            xt[:, kt, jt * P:(jt + 1) * P],
                            wv_sb[:, kt, :],
                            start=(kt == 0),
                            stop=(kt == KT - 1),
                        )
                    nc.vector.tensor_tensor(
                        vh[:, jt, :, 0:64],
                        ps.rearrange("p (l d) -> p l d", l=HG),
                        bv_sb.rearrange("p (l d) -> p l d", l=HG),
                        mybir.AluOpType.add,
                    )
                for src, wsb, dst, bcol in ((xk, wk_sb, khT, 2), (xq, wq_sb, qhT, 0)):
                    xt = xp.tile([P, KT, L], BF16, tag="xin")
                    # per-kt chunk DMAs: matmuls start as soon as the first
                    # chunk lands instead of waiting for the full tensor
                    for kt in range(KT):
                        nc.sync.dma_start(xt[:, kt, :],
                                          src.ap()[kt * P:(kt + 1) * P, :])
                    # kt-outer per hp: one LDWEIGHTS feeds 4 back-to-back
                    # matmuls; 4 psum tiles accumulate in parallel
                    for hp in range(2):
                        pss = {}
                        for ic in range(L // 512):
                            pss[ic] = ps1.tile([P, 512], F32, tag="ps1",
                                               name=f"ps1_{ic}")
                        for kt in range(KT):
                            for ic in range(L // 512):
                                nc.tensor.matmul(
                                    pss[ic],
                                    wsb[:, kt, hp * P:(hp + 1) * P],
                                    xt[:, kt, ic * 512:(ic + 1) * 512],
                                    start=(kt == 0),
                                    stop=(kt == KT - 1),
                                )
                        for ic in range(L // 512):
                            nc.vector.tensor_scalar_add(
                                dst[:, hp, ic * 512:(ic + 1) * 512],
                                pss[ic],
                                bqk_sb[:, bcol + hp:bcol + hp + 1],
                            )

            # ---- Stage 2: attention ----
            # The two heads of a pair are interleaved so adjacent K=64
            # matmuls target different PE row-groups and run concurrently.
            with (
                tc.tile_pool(name="psA", bufs=4, space="PSUM") as psAp,
                tc.tile_pool(name="psT", bufs=2, space="PSUM") as psTp,
                tc.tile_pool(name="psO", bufs=2, space="PSUM") as psOp,
            ):
                itc = 0
                for it in range(NIT):
                    nkt = it + 1               # valid 128-key tiles
                    W = nkt * P                # valid band width (r cols)
                    ncj = (W + 511) // 512     # 512-wide chunks (last partial)
                    i0 = it * P
                    r_lo = L - P - i0          # first relative index in band
                    for hp in range(2):
                        par = itc & 1
                        itc += 1
                        q_stat = [qhT[64 * hl:64 * hl + 64, hp, i0:i0 + P]
                                  for hl in (0, 1)]
                        xse = [xse_pp[hl][par] for hl in (0, 1)]

                        # exp(Srel/8) bands, [query on partitions, r on free]
                        for cs in range(ncj):
                            n = min(512, W - cs * 512)
                            for hl in (0, 1):
                                pb = 64 * hl
                                ps = psAp.tile([P, 512], F32, tag="psA")
                                nc.tensor.matmul(
                                    ps[:, :n],
                                    q_stat[hl],
                                    eT_sb[pb:pb + 64, hp,
                                          r_lo + cs * 512:r_lo + cs * 512 + n],
                                    start=True, stop=True,
                                )
                                nc.scalar.activation(
                                    xse[hl][:, cs * 512:cs * 512 + n], ps[:, :n],
                                    mybir.ActivationFunctionType.Exp, scale=SCALE,
                                )

                        # exp(QK^T/8), trimmed to the valid band
                        pqk = [wp.tile([P, 2048], BF16, tag=f"pqk{hl}", name=f"pqk{hl}")
                               for hl in (0, 1)]
                        for jc in range(ncj):
                            n = min(512, W - jc * 512)
                            for hl in (0, 1):
                                pb = 64 * hl
                                ps = psAp.tile([P, 512], F32, tag="psA")
                                nc.tensor.matmul(
                                    ps[:, :n],
                                    q_stat[hl],
                                    khT[pb:pb + 64, hp, jc * 512:jc * 512 + n],
                                    start=True, stop=True,
                                )
                                nc.scalar.activation(
                                    pqk[hl][:, jc * 512:jc * 512 + n], ps[:, :n],
                                    mybir.ActivationFunctionType.Exp, scale=SCALE,
                                )

                        # skew (SBUF->SBUF diagonal DMA), then the fused
                        # P = pqk * xsk multiply whose accumulator output IS
                        # the softmax denominator (per-query row sum).
                        rec_pack = sp.tile([P, 2], F32, tag="recp")
                        pm = []
                        for hl in (0, 1):
                            xsk = wp.tile([P, 2048], BF16, tag=f"xsk{hl}")
                            row_len = xse[hl].ap[0][0]
                            diag = bass.AP(
                                xse[hl].tensor, xse[hl].offset + 127,
                                [[row_len - 1, P], [1, W]],
                            )
                            nc.sync.dma_start(xsk[:, :W], diag)
                            pmt = wp.tile([P, 2048], BF16, tag=f"pm{hl}")
                            nc.vector.tensor_tensor_reduce(
                                out=pmt[:, :W],
                                in0=pqk[hl][:, :W],
                                in1=xsk[:, :W],
                                scale=1.0,
                                scalar=0.0,
                                op0=mybir.AluOpType.mult,
                                op1=mybir.AluOpType.add,
                                accum_out=rec_pack[:, hl:hl + 1],
                            )
                            pm.append(pmt)

                        # PV into column-half psum tiles: head hl occupies
                        # psum partitions [64*hl, 64*hl+64). psot cols
                        # [0:128] hold PV, [128:256] the replicated
                        # reciprocals, [256:384] the transposed rec rows --
                        # all one psum bank.
                        psot = psOp.tile([P, 384], F32, tag="psO")
                        pso = psot[:, 0:P]
                        for jb in range(ncj):
                            w_jb = min(512, W - jb * 512)
                            ntk = (w_jb + P - 1) // P
                            for hl in (0, 1):
                                pb = 64 * hl
                                lh = 2 * hp + hl
                                pst = psTp.tile([P, 512], BF16, tag="psT")
                                for t in range(ntk):
                                    nc.tensor.transpose(
                                        pst[:, t * P:(t + 1) * P],
                                        pm[hl][:, jb * 512 + t * P:
                                               jb * 512 + (t + 1) * P],
                                        ident,
                                    )
                                pts = sp.tile([P, 512], BF16, tag="pts")
                                nc.vector.tensor_copy(pts[:, :w_jb],
                                                      pst[:, :w_jb])
                                for t in range(ntk):
                                    jt = jb * 4 + t
                                    nc.tensor.matmul(
                                        pso[pb:pb + 64, :],
                                        vh[:, jt, lh, 0:64],
                                        pts[:, t * P:(t + 1) * P],
                                        start=(jt == 0),
                                        stop=(jt == nkt - 1),
                                        skip_group_check=True,
                                    )

                        # per-query reciprocal -> partition-replicated psum:
                        # transpose rec [P,2] to a [2,128] psum row pair,
                        # copy to SBUF, then a mask2-stationary matmul
                        # replicates each head's row over its 64 partitions.
                        rec_f = sp.tile([P, 2], F32, tag="recf")
                        nc.vector.reciprocal_approx_fast(
                            out=rec_f, in_=rec_pack)
                        nc.tensor.transpose(psot[0:2, 256:384], rec_f,
                                            ident_f32)
                        rrow = sp.tile([2, P], F32, tag="rrow")
                        nc.vector.tensor_copy(rrow, psot[0:2, 256:384])
                        nc.tensor.matmul(psot[:, 128:256], mask2, rrow,
                                         start=True, stop=True)
                        rrec = sp.tile([P, P], F32, tag="rrec")
                        nc.vector.tensor_copy(rrec, psot[:, 128:256])

                        # normalize both heads at once; writes land directly
                        # in the packed outT partition halves
                        nc.vector.tensor_tensor(
                            outT[:, hp, i0:i0 + P], pso, rrec,
                            mybir.AluOpType.mult,
                        )

            # ---- Stage 3: output projection (partial: this head group) ----
            with tc.tile_pool(name="ps3", bufs=2, space="PSUM") as ps3:
                for it in range(NIT):
                    pss3 = [ps3.tile([P, 512], F32, tag=f"ps3{mc}",
                                     name=f"ps3{mc}")
                            for mc in range(DM // 512)]
                    for hp in range(2):
                        for mc in range(DM // 512):
                            nc.tensor.matmul(
                                pss3[mc],
                                outT[:, hp, it * P:(it + 1) * P],
                                wo_sb[:, hp, mc * 512:(mc + 1) * 512],
                                start=(hp == 0),
                                stop=(hp == 1),
                            )
                    for mc in range(DM // 512):
                        osb = sp.tile([P, 512], F32, tag="osb")
                        nc.vector.tensor_tensor(
                            osb, pss3[mc], bo_sb[:, mc * 512:(mc + 1) * 512],
                            mybir.AluOpType.add,
                        )
                        nc.sync.dma_start(
                            out.ap()[it * P:(it + 1) * P, mc * 512:(mc + 1) * 512], osb
                        )
    nc.compile()
    return nc


def _prep_inputs(q, k, v, Wq, bq, Wk, bk, Wv, bv, Wo, bo, E):
    """Build the 8 per-core input maps (host-side shard + cast)."""
    in_maps = []
    for core in range(NCORES):
        b, g = core // HG, core % HG
        cols = slice(g * HG * D, (g + 1) * HG * D)  # this group's 256 cols
        # eT/wo packing: [hp, 64*hl + d, .]
        eTg = np.empty((2, P, L), NPBF16)
        wog = np.empty((2, P, DM), NPBF16)
        for hp in range(2):
            for hl in range(2):
                h = g * HG + 2 * hp + hl
                eTg[hp, 64 * hl:64 * hl + 64, :] = E[:, h * D:(h + 1) * D].T.astype(NPBF16)
                wog[hp, 64 * hl:64 * hl + 64, :] = Wo[h * D:(h + 1) * D, :].astype(NPBF16)
        bqk_a = np.empty((P, 4), np.float32)
        for hp in range(2):
            bqk_a[:, hp] = bq[g * HG * D + hp * P:g * HG * D + (hp + 1) * P]
            bqk_a[:, 2 + hp] = bk[g * HG * D + hp * P:g * HG * D + (hp + 1) * P]
        bo_full = bo if g == 0 else np.zeros_like(bo)
        mask2_a = np.zeros((2, P), np.float32)
        mask2_a[0, 0:64] = 1.0
        mask2_a[1, 64:128] = 1.0
        in_maps.append({
            "mask2": mask2_a,
            "xq": np.ascontiguousarray(q[b].T).astype(NPBF16),
            "xk": np.ascontiguousarray(k[b].T).astype(NPBF16),
            "xv": np.ascontiguousarray(v[b].T).astype(NPBF16),
            "wq": np.ascontiguousarray(Wq[:, cols]).astype(NPBF16),
            "wk": np.ascontiguousarray(Wk[:, cols]).astype(NPBF16),
            "wv": np.ascontiguousarray(Wv[:, cols]).astype(NPBF16),
            "wo": wog,
            "eT": eTg,
            "bqk": bqk_a,
            "bv": np.ascontiguousarray(
                np.broadcast_to(bv[None, cols], (P, 2 * P))).astype(np.float32),
            "bo": np.ascontiguousarray(
                np.broadcast_to(bo_full[None, :], (P, DM))).astype(np.float32),
        })
    return in_maps


def _reference_numpy(q, k, v, mask, Wq, bq, Wk, bk, Wv, bv, Wo, bo, E):
    """Exact fallback for non-causal masks (never hit in practice)."""
    def split_heads(x):
        return np.moveaxis(x.reshape(*x.shape[:-1], H, D), -2, -3)
    qh = split_heads(q @ Wq + bq)
    kh = split_heads(k @ Wk + bk)
    vv = split_heads(v @ Wv + bv)
    eh = split_heads(E)
    QKt = np.einsum("bhqd,bhkd->bhqk", qh, kh)
    X = np.einsum("bhqd,hkd->bhqk", qh, eh)
    pad = np.pad(X, [(0, 0)] * 3 + [(1, 0)])
    s = pad.reshape(B, H, -1)[:, :, L:].reshape(B, H, L, L)
    logits = (QKt + s) / np.sqrt(D) + mask * -1e9
    m = logits.max(-1, keepdims=True)
    p = np.exp(logits - m)
    p /= p.sum(-1, keepdims=True)
    o = np.einsum("bhqk,bhkd->bhqd", p, vv)
    o = np.moveaxis(o, -3, -2).reshape(B, L, DM)
    return (o @ Wo + bo).astype(np.float32)


def benchmark(inputs, iters=20):
    """Amortized wall-clock of the sharded NEFF execution (device-resident
    inputs, back-to-back async dispatch). Returns est. ns per execution."""
    global _PROG
    import time as _time
    import jax
    from jax.sharding import Mesh, PartitionSpec
    from jax.experimental.shard_map import shard_map
    import concourse.bass2jax as b2j
    import concourse.mybir as mb

    if _PROG is None:
        _PROG = build_program()
    nc = _PROG
    args = {n: np.asarray(inputs[n], np.float32)
            for n in ("q", "k", "v", "Wq", "bq", "Wk", "bk", "Wv", "bv",
                      "Wo", "bo", "E")}
    in_maps = _prep_inputs(**args)
    b2j.install_neuronx_cc_hook()

    partition_name = (nc.partition_id_tensor.name
                      if nc.partition_id_tensor else None)
    in_names, out_names, out_avals, zero_outs = [], [], [], []
    for alloc in nc.m.functions[0].allocations:
        if not isinstance(alloc, mb.MemoryLocationSet):
            continue
        name = alloc.memorylocations[0].name
        if alloc.kind == "ExternalInput":
            if name != partition_name:
                in_names.append(name)
        elif alloc.kind == "ExternalOutput":
            out_names.append(name)
            shape = tuple(alloc.tensor_shape)
            dtype = mb.dt.np(alloc.dtype)
            out_avals.append(jax.core.ShapedArray(shape, dtype))
            zero_outs.append(np.zeros(shape, dtype))
    n_params = len(in_names)
    n_outs = len(out_avals)
    all_names = in_names + out_names
    if partition_name is not None:
        all_names = all_names + [partition_name]

    def _body(*fargs):
        operands = list(fargs)
        if partition_name is not None:
            operands.append(b2j.partition_id_tensor())
        outs = b2j._bass_exec_p.bind(
            *operands, out_avals=tuple(out_avals), in_names=tuple(all_names),
            out_names=tuple(out_names), lowering_input_output_aliases=(),
            sim_require_finite=True, sim_require_nnan=True, nc=nc)
        return tuple(outs)

    devices = jax.devices()[:NCORES]
    mesh = Mesh(np.asarray(devices), ("core",))
    in_specs = (PartitionSpec("core"),) * (n_params + n_outs)
    out_specs = (PartitionSpec("core"),) * n_outs
    sharded = jax.jit(
        shard_map(_body, mesh=mesh, in_specs=in_specs, out_specs=out_specs,
                  check_rep=False),
        keep_unused=True)

    concat_in = [np.concatenate([np.asarray(in_maps[c][n])
                                 for c in range(NCORES)], axis=0)
                 for n in in_names]
    dev_in = [jax.device_put(a) for a in concat_in]
    concat_zero = [np.concatenate([z] * NCORES, axis=0) for z in zero_outs]

    dev_zero = [jax.device_put(z) for z in concat_zero]
    # warmup (compiles / caches)
    outs = sharded(*dev_in, *dev_zero)
    jax.block_until_ready(outs)

    t0 = _time.perf_counter()
    results = []
    for _ in range(iters):
        results.append(sharded(*dev_in, *dev_zero))
    jax.block_until_ready(results)
    t1 = _time.perf_counter()
    return (t1 - t0) / iters * 1e9


def kernel(**inputs):
    global _PROG, LAST_EXEC_NS
    args = {n: np.asarray(inputs[n], np.float32)
            for n in ("q", "k", "v", "Wq", "bq", "Wk", "bk", "Wv", "bv",
                      "Wo", "bo", "E")}
    mask = np.asarray(inputs["mask"], np.float32)

    causal = np.array_equal(mask, np.triu(np.ones((L, L), np.float32), k=1))
    if not causal:
        return _reference_numpy(mask=mask, **args)

    if _PROG is None:
        _PROG = build_program()
    in_maps = _prep_inputs(**args)
    trace = os.environ.get("KERNEL_TRACE", "0") == "1"
    try:
        res = run_bass_kernel_spmd(_PROG, in_maps, core_ids=list(range(NCORES)),
                                   trace=trace)
    except ModuleNotFoundError:
        # axon NTFF profiling hook unavailable in this container
        res = run_bass_kernel_spmd(_PROG, in_maps, core_ids=list(range(NCORES)),
                                   trace=False)
    LAST_EXEC_NS = res.exec_time_ns
    globals()["LAST_RESULTS"] = res

    full = np.zeros((B, L, DM), np.float32)
    for core in range(NCORES):
        full[core // HG] += res.results[core]["out"]
    return full



# revision 34
# speedup vs baseline: 63.3783x; 63.3783x over previous
"""Trainium2 Bass kernel for nn_MultiHeadAttention_21251498181338.

Music-Transformer-style MHA with relative position embeddings (Huang et al.
skew trick), B=2, L=2048, D=1024, H=16, causal mask.

Sharding: 8 cores = 2 batches x 4-head groups (tensor parallel per head).
Each core computes q/k/v projections for its 4 heads, causal attention with
relative position logits, and a partial output projection (Wo row-split).
Partials are summed on the host during unshard.

Device-side structure (per core):
  - Projections produce qh^T/kh^T in [head-depth on partitions] layout and
    vh in [keys on partitions] layout, so no transposes are needed anywhere
    except for the attention probabilities themselves.
  - P = exp(QK^T/8) * exp(Srel/8): the additive logit split is computed
    multiplicatively so the relative-position term can be skew-aligned
    independently of QK^T.
  - The skew is a single SBUF->SBUF DMA per (head, q-tile) using a flat
    access pattern with partition step (row_len - 1): row i is read with a
    column offset of -i, which is exactly the Huang et al. pad/reshape
    trick. Columns beyond the valid relative-index range are zeroed, which
    also implements the causal mask for free (P = Pqk * 0 = 0 there).
  - PV uses TensorE transposes of P tiles. The softmax denominators come
    for free from the fused multiply+reduce (tensor_tensor_reduce): the
    per-query row sums accumulate on the vector engine during the
    P = exp(QK)*exp(Srel) multiply, so no denominator matmuls are needed.
    The per-query reciprocals are turned into a partition-replicated
    [128, 128] tile via a DVE 32x32 stream-transpose plus two tiny
    SBUF->SBUF broadcast DMAs.
  - The two heads of each pair interleave their K=64 matmuls (different PE
    row-groups run concurrently) and share [128, P] psum tiles for PV via
    tile_position column halves, so both heads normalize in one op and
    land directly in the packed outT layout.
  - The attention output appears transposed [depth, queries], which is
    exactly the stationary-operand layout the output projection needs.
"""

import os
import sys

sys.path.insert(0, "/opt/trn_rl_repo")

import numpy as np
import ml_dtypes

import concourse.bass as bass
import concourse.mybir as mybir
import concourse.tile as tile
from concourse import bacc
from concourse.bass_utils import run_bass_kernel_spmd
from concourse.masks import make_identity

BF16 = mybir.dt.bfloat16
F32 = mybir.dt.float32
NPBF16 = ml_dtypes.bfloat16

USE_TTR = os.environ.get("K_TTR", "0") == "1"
USE_CHAIN = os.environ.get("K_CHAIN", "1") == "1"

B, L, DM, H, D = 2, 2048, 1024, 16, 64
HG = 4            # heads per core (head group)
NCORES = 8
P = 128
KT = DM // P      # 8 contraction tiles for projections
NIT = L // P      # 16 query tiles
SCALE = 1.0 / np.sqrt(D)  # 0.125

LAST_EXEC_NS = None

_PROG = None


def _ncj(it):
    # number of 512-wide key chunks for query tile `it` (causal)
    return it // 4 + 1


def build_program():
    nc = bacc.Bacc(
        "TRN2",
        target_bir_lowering=False,
        debug=False,
        enable_asserts=False,
        num_devices=NCORES,
    )

    # ---- External I/O ----
    xq = nc.dram_tensor("xq", [DM, L], BF16, kind="ExternalInput")  # q[b].T
    xk = nc.dram_tensor("xk", [DM, L], BF16, kind="ExternalInput")
    xv = nc.dram_tensor("xv", [DM, L], BF16, kind="ExternalInput")
    wq = nc.dram_tensor("wq", [DM, 2 * P], BF16, kind="ExternalInput")  # group cols
    wk = nc.dram_tensor("wk", [DM, 2 * P], BF16, kind="ExternalInput")
    wv = nc.dram_tensor("wv", [DM, 2 * P], BF16, kind="ExternalInput")
    wo = nc.dram_tensor("wo", [2, P, DM], BF16, kind="ExternalInput")  # [hp, 2h*64, dm]
    eT = nc.dram_tensor("eT", [2, P, L], BF16, kind="ExternalInput")   # [hp, 2h*64, r]
    bqk = nc.dram_tensor("bqk", [P, 4], F32, kind="ExternalInput")  # cols 0:2 bq, 2:4 bk
    mask2_d = nc.dram_tensor("mask2", [2, P], BF16, kind="ExternalInput")
    bv_t = nc.dram_tensor("bv", [P, 2 * P], F32, kind="ExternalInput")  # row-replicated
    bo_t = nc.dram_tensor("bo", [P, DM], F32, kind="ExternalInput")     # row-replicated
    out = nc.dram_tensor("out", [L, DM], F32, kind="ExternalOutput")

    with tile.TileContext(nc) as tc:
        with (
            tc.tile_pool(name="persist", bufs=1) as pp,
            tc.tile_pool(name="work", bufs=2) as wp,
            tc.tile_pool(name="small", bufs=4) as sp,
        ):
            # ---- persistent SBUF tensors ----
            ident = pp.tile([P, P], BF16)
            make_identity(nc, ident)

            wq_sb = pp.tile([P, KT, 2 * P], BF16)
            nc.sync.dma_start(wq_sb, wq.ap().rearrange("(t p) c -> p t c", p=P))
            wk_sb = pp.tile([P, KT, 2 * P], BF16)
            nc.sync.dma_start(wk_sb, wk.ap().rearrange("(t p) c -> p t c", p=P))
            wv_sb = pp.tile([P, KT, 2 * P], BF16)
            nc.sync.dma_start(wv_sb, wv.ap().rearrange("(t p) c -> p t c", p=P))
            wo_sb = pp.tile([P, 2, DM], BF16)
            nc.sync.dma_start(wo_sb, wo.ap().rearrange("h p m -> p h m"))
            eT_sb = pp.tile([P, 2, L], BF16)
            nc.sync.dma_start(eT_sb, eT.ap().rearrange("h p r -> p h r"))
            bqk_sb = pp.tile([P, 4], F32)
            nc.sync.dma_start(bqk_sb, bqk.ap())
            bv_sb = pp.tile([P, 2 * P], F32)
            nc.sync.dma_start(bv_sb, bv_t.ap())
            bo_sb = pp.tile([P, DM], F32)
            nc.sync.dma_start(bo_sb, bo_t.ap())

            qhT = pp.tile([P, 2, L], BF16)   # [64*hl+d, hp, i]
            khT = pp.tile([P, 2, L], BF16)
            vh = pp.tile([P, NIT, HG, 66], BF16)  # [j in tile, jt, local head, d|1|pad]
            outT = pp.tile([P, 2, L], BF16)  # [64*hl+d, hp, i]

            # persistent exp(Srel) band buffers, zeroed ONCE: each use writes
            # exp values into [0:W] and the skew DMA reads zeros beyond W.
            # W grows monotonically with it, so the zero tail stays clean.
            XSE_W = 2176  # 2048 max band + 128 skew overhang
            xse_pp = [[pp.tile([P, XSE_W], BF16, name=f"xse{hl}_{par}")
                       for par in range(2)] for hl in range(2)]
            for hl in range(2):
                for par in range(2):
                    nc.gpsimd.memset(xse_pp[hl][par], 0.0)
            # head-half selector used to replicate each head's reciprocal
            # row across its 64 psum partitions (bf16: 0/1 exact).
            mask2 = pp.tile([2, P], BF16)
            nc.sync.dma_start(mask2, mask2_d.ap())
            if not (USE_TTR and USE_CHAIN):
                ones64 = pp.tile([P, 64], BF16)
                nc.gpsimd.memset(ones64, 1.0)

            # ---- Stage 1: projections ----
            with (
                tc.tile_pool(name="xin", bufs=2) as xp,
                tc.tile_pool(name="ps1", bufs=4, space="PSUM") as ps1,
            ):
                # v projection first (vh used last; q/k streams lead into
                # stage 2 with the PE warm)
                xt = xp.tile([P, KT, L], BF16, tag="xin")
                for kt in range(KT):
                    nc.sync.dma_start(xt[:, kt, :],
                                      xv.ap()[kt * P:(kt + 1) * P, :])
                for jt in range(NIT):
                    ps = ps1.tile([P, 2 * P], F32, tag="psv")
                    for kt in range(KT):
                        nc.tensor.matmul(
                            ps,
                            xt[:, kt, jt * P:(jt + 1) * P],
                            wv_sb[:, kt, :],
                            start=(kt == 0),
                            stop=(kt == KT - 1),
                        )
                    nc.vector.tensor_tensor(
                        vh[:, jt, :, 0:64],
                        ps.rearrange("p (l d) -> p l d", l=HG),
                        bv_sb.rearrange("p (l d) -> p l d", l=HG),
                        mybir.AluOpType.add,
                    )
                for src, wsb, dst, bcol in ((xk, wk_sb, khT, 2), (xq, wq_sb, qhT, 0)):
                    xt = xp.tile([P, KT, L], BF16, tag="xin")
                    # per-kt chunk DMAs: matmuls start as soon as the first
                    # chunk lands instead of waiting for the full tensor
                    for kt in range(KT):
                        nc.sync.dma_start(xt[:, kt, :],
                                          src.ap()[kt * P:(kt + 1) * P, :])
                    # kt-outer per hp: one LDWEIGHTS feeds 4 back-to-back
                    # matmuls; 4 psum tiles accumulate in parallel
                    for hp in range(2):
                        pss = {}
                        for ic in range(L // 512):
                            pss[ic] = ps1.tile([P, 512], F32, tag="ps1",
                                               name=f"ps1_{ic}")
                        for kt in range(KT):
                            for ic in range(L // 512):
                                nc.tensor.matmul(
                                    pss[ic],
                                    wsb[:, kt, hp * P:(hp + 1) * P],
                                    xt[:, kt, ic * 512:(ic + 1) * 512],
                                    start=(kt == 0),
                                    stop=(kt == KT - 1),
                                )
                        for ic in range(L // 512):
                            nc.vector.tensor_scalar_add(
                                dst[:, hp, ic * 512:(ic + 1) * 512],
                                pss[ic],
                                bqk_sb[:, bcol + hp:bcol + hp + 1],
                            )

            # ---- Stage 2: attention ----
            # The two heads of a pair are interleaved so adjacent K=64
            # matmuls target different PE row-groups and run concurrently.
            with (
                tc.tile_pool(name="psA", bufs=4 if (USE_TTR and USE_CHAIN)
                             else 3, space="PSUM") as psAp,
                tc.tile_pool(name="psT", bufs=2, space="PSUM") as psTp,
                tc.tile_pool(name="psO", bufs=2, space="PSUM") as psOp,
                tc.tile_pool(name="psD", bufs=1, space="PSUM") as psDp,
            ):
                itc = 0
                for it in range(NIT):
                    nkt = it + 1               # valid 128-key tiles
                    W = nkt * P                # valid band width (r cols)
                    ncj = (W + 511) // 512     # 512-wide chunks (last partial)
                    i0 = it * P
                    r_lo = L - P - i0          # first relative index in band
                    for hp in range(2):
                        par = itc & 1
                        itc += 1
                        q_stat = [qhT[64 * hl:64 * hl + 64, hp, i0:i0 + P]
                                  for hl in (0, 1)]
                        xse = [xse_pp[hl][par] for hl in (0, 1)]

                        # exp(Srel/8) bands, [query on partitions, r on free]
                        for cs in range(ncj):
                            n = min(512, W - cs * 512)
                            for hl in (0, 1):
                                pb = 64 * hl
                                ps = psAp.tile([P, 512], F32, tag="psA")
                                nc.tensor.matmul(
                                    ps[:, :n],
                                    q_stat[hl],
                                    eT_sb[pb:pb + 64, hp,
                                          r_lo + cs * 512:r_lo + cs * 512 + n],
                                    start=True, stop=True,
                                )
                                nc.scalar.activation(
                                    xse[hl][:, cs * 512:cs * 512 + n], ps[:, :n],
                                    mybir.ActivationFunctionType.Exp, scale=SCALE,
                                )

                        # exp(QK^T/8), trimmed to the valid band
                        pqk = [wp.tile([P, 2048], BF16, tag=f"pqk{hl}", name=f"pqk{hl}")
                               for hl in (0, 1)]
                        for jc in range(ncj):
                            n = min(512, W - jc * 512)
                            for hl in (0, 1):
                                pb = 64 * hl
                                ps = psAp.tile([P, 512], F32, tag="psA")
                                nc.tensor.matmul(
                                    ps[:, :n],
                                    q_stat[hl],
                                    khT[pb:pb + 64, hp, jc * 512:jc * 512 + n],
                                    start=True, stop=True,
                                )
                                nc.scalar.activation(
                                    pqk[hl][:, jc * 512:jc * 512 + n], ps[:, :n],
                                    mybir.ActivationFunctionType.Exp, scale=SCALE,
                                )

                        # skew (SBUF->SBUF diagonal DMA), then the fused
                        # P = pqk * xsk multiply whose accumulator output IS
                        # the softmax denominator (per-query row sum).
                        rec_pack = sp.tile([P, 2], F32, tag="recp")
                        pm = []
                        for hl in (0, 1):
                            xsk = wp.tile([P, 2048], BF16, tag=f"xsk{hl}")
                            row_len = xse[hl].ap[0][0]
                            diag = bass.AP(
                                xse[hl].tensor, xse[hl].offset + 127,
                                [[row_len - 1, P], [1, W]],
                            )
                            nc.sync.dma_start(xsk[:, :W], diag)
                            pmt = wp.tile([P, 2048], BF16, tag=f"pm{hl}")
                            if USE_TTR:
                                nc.vector.tensor_tensor_reduce(
                                    out=pmt[:, :W],
                                    in0=pqk[hl][:, :W],
                                    in1=xsk[:, :W],
                                    scale=1.0,
                                    scalar=0.0,
                                    op0=mybir.AluOpType.mult,
                                    op1=mybir.AluOpType.add,
                                    accum_out=rec_pack[:, hl:hl + 1],
                                )
                            else:
                                nc.vector.tensor_tensor(
                                    pmt[:, :W], pqk[hl][:, :W], xsk[:, :W],
                                    mybir.AluOpType.mult,
                                )
                            pm.append(pmt)

                        # PV into column-half psum tiles: head hl occupies
                        # psum partitions [64*hl, 64*hl+64). NOTE: a psum
                        # bank tolerates only ONE open accumulation group --
                        # pso gets its bank to itself.
                        pso = psOp.tile([P, P], F32, tag="psO")
                        psd = (None if (USE_TTR and USE_CHAIN)
                               else psDp.tile([P, P], F32, tag="psD"))
                        for jb in range(ncj):
                            w_jb = min(512, W - jb * 512)
                            ntk = (w_jb + P - 1) // P
                            for hl in (0, 1):
                                pb = 64 * hl
                                lh = 2 * hp + hl
                                pst = psTp.tile([P, 512], BF16, tag="psT")
                                for t in range(ntk):
                                    nc.tensor.transpose(
                                        pst[:, t * P:(t + 1) * P],
                                        pm[hl][:, jb * 512 + t * P:
                                               jb * 512 + (t + 1) * P],
                                        ident,
                                    )
                                pts = sp.tile([P, 512], BF16, tag="pts")
                                nc.vector.tensor_copy(pts[:, :w_jb],
                                                      pst[:, :w_jb])
                                for t in range(ntk):
                                    jt = jb * 4 + t
                                    nc.tensor.matmul(
                                        pso[pb:pb + 64, :],
                                        vh[:, jt, lh, 0:64],
                                        pts[:, t * P:(t + 1) * P],
                                        start=(jt == 0),
                                        stop=(jt == nkt - 1),
                                        skip_group_check=True,
                                    )
                                    if psd is not None:
                                        nc.tensor.matmul(
                                            psd[pb:pb + 64, :],
                                            ones64,
                                            pts[:, t * P:(t + 1) * P],
                                            start=(jt == 0),
                                            stop=(jt == nkt - 1),
                                            skip_group_check=True,
                                        )

                        # per-query reciprocal -> partition-replicated psum:
                        # transpose rec [P,2] to a [2,128] psum row pair,
                        # copy to SBUF, then a mask2-stationary matmul
                        # replicates each head's row over its 64 partitions.
                        rrec = sp.tile([P, P], F32, tag="rrec")
                        if USE_TTR and USE_CHAIN:
                            rec_f = sp.tile([P, 2], F32, tag="recf")
                            nc.vector.reciprocal_approx_fast(
                                out=rec_f, in_=rec_pack)
                            rec_b = sp.tile([P, 2], BF16, tag="recb")
                            nc.vector.tensor_copy(rec_b, rec_f)
                            psrow = psTp.tile([2, P], BF16, tag="psT")
                            nc.tensor.transpose(psrow, rec_b, ident)
                            rrow = sp.tile([2, P], BF16, tag="rrow")
                            nc.vector.tensor_copy(rrow, psrow)
                            psrec = psTp.tile([P, P], F32, tag="psT")
                            nc.tensor.matmul(psrec, mask2, rrow,
                                             start=True, stop=True)
                            nc.vector.tensor_copy(rrec, psrec)
                        else:
                            nc.vector.reciprocal_approx_fast(
                                out=rrec, in_=psd)

                        # normalize both heads at once; writes land directly
                        # in the packed outT partition halves
                        nc.vector.tensor_tensor(
                            outT[:, hp, i0:i0 + P], pso, rrec,
                            mybir.AluOpType.mult,
                        )

            # ---- Stage 3: output projection (partial: this head group) ----
            with tc.tile_pool(name="ps3", bufs=2, space="PSUM") as ps3:
                for it in range(NIT):
                    pss3 = [ps3.tile([P, 512], F32, tag=f"ps3{mc}",
                                     name=f"ps3{mc}")
                            for mc in range(DM // 512)]
                    for hp in range(2):
                        for mc in range(DM // 512):
                            nc.tensor.matmul(
                                pss3[mc],
                                outT[:, hp, it * P:(it + 1) * P],
                                wo_sb[:, hp, mc * 512:(mc + 1) * 512],
                                start=(hp == 0),
                                stop=(hp == 1),
                            )
                    for mc in range(DM // 512):
                        osb = sp.tile([P, 512], F32, tag="osb")
                        nc.vector.tensor_tensor(
                            osb, pss3[mc], bo_sb[:, mc * 512:(mc + 1) * 512],
                            mybir.AluOpType.add,
                        )
                        nc.sync.dma_start(
                            out.ap()[it * P:(it + 1) * P, mc * 512:(mc + 1) * 512], osb
                        )
    nc.compile()
    return nc


def _prep_inputs(q, k, v, Wq, bq, Wk, bk, Wv, bv, Wo, bo, E):
    """Build the 8 per-core input maps (host-side shard + cast)."""
    in_maps = []
    for core in range(NCORES):
        b, g = core // HG, core % HG
        cols = slice(g * HG * D, (g + 1) * HG * D)  # this group's 256 cols
        # eT/wo packing: [hp, 64*hl + d, .]
        eTg = np.empty((2, P, L), NPBF16)
        wog = np.empty((2, P, DM), NPBF16)
        for hp in range(2):
            for hl in range(2):
                h = g * HG + 2 * hp + hl
                eTg[hp, 64 * hl:64 * hl + 64, :] = E[:, h * D:(h + 1) * D].T.astype(NPBF16)
                wog[hp, 64 * hl:64 * hl + 64, :] = Wo[h * D:(h + 1) * D, :].astype(NPBF16)
        bqk_a = np.empty((P, 4), np.float32)
        for hp in range(2):
            bqk_a[:, hp] = bq[g * HG * D + hp * P:g * HG * D + (hp + 1) * P]
            bqk_a[:, 2 + hp] = bk[g * HG * D + hp * P:g * HG * D + (hp + 1) * P]
        bo_full = bo if g == 0 else np.zeros_like(bo)
        mask2_a = np.zeros((2, P), np.float32)
        mask2_a[0, 0:64] = 1.0
        mask2_a[1, 64:128] = 1.0
        in_maps.append({
            "mask2": mask2_a,
            "xq": np.ascontiguousarray(q[b].T).astype(NPBF16),
            "xk": np.ascontiguousarray(k[b].T).astype(NPBF16),
            "xv": np.ascontiguousarray(v[b].T).astype(NPBF16),
            "wq": np.ascontiguousarray(Wq[:, cols]).astype(NPBF16),
            "wk": np.ascontiguousarray(Wk[:, cols]).astype(NPBF16),
            "wv": np.ascontiguousarray(Wv[:, cols]).astype(NPBF16),
            "wo": wog,
            "eT": eTg,
            "bqk": bqk_a,
            "bv": np.ascontiguousarray(
                np.broadcast_to(bv[None, cols], (P, 2 * P))).astype(np.float32),
            "bo": np.ascontiguousarray(
                np.broadcast_to(bo_full[None, :], (P, DM))).astype(np.float32),
        })
    return in_maps


def _reference_numpy(q, k, v, mask, Wq, bq, Wk, bk, Wv, bv, Wo, bo, E):
    """Exact fallback for non-causal masks (never hit in practice)."""
    def split_heads(x):
        return np.moveaxis(x.reshape(*x.shape[:-1], H, D), -2, -3)
    qh = split_heads(q @ Wq + bq)
    kh = split_heads(k @ Wk + bk)
    vv = split_heads(v @ Wv + bv)
    eh = split_heads(E)
    QKt = np.einsum("bhqd,bhkd->bhqk", qh, kh)
    X = np.einsum("bhqd,hkd->bhqk", qh, eh)
    pad = np.pad(X, [(0, 0)] * 3 + [(1, 0)])
    s = pad.reshape(B, H, -1)[:, :, L:].reshape(B, H, L, L)
    logits = (QKt + s) / np.sqrt(D) + mask * -1e9
    m = logits.max(-1, keepdims=True)
    p = np.exp(logits - m)
    p /= p.sum(-1, keepdims=True)
    o = np.einsum("bhqk,bhkd->bhqd", p, vv)
    o = np.moveaxis(o, -3, -2).reshape(B, L, DM)
    return (o @ Wo + bo).astype(np.float32)


def benchmark(inputs, iters=20):
    """Amortized wall-clock of the sharded NEFF execution (device-resident
    inputs, back-to-back async dispatch). Returns est. ns per execution."""
    global _PROG
    import time as _time
    import jax
    from jax.sharding import Mesh, PartitionSpec
    from jax.experimental.shard_map import shard_map
    import concourse.bass2jax as b2j
    import concourse.mybir as mb

    if _PROG is None:
        _PROG = build_program()
    nc = _PROG
    args = {n: np.asarray(inputs[n], np.float32)
            for n in ("q", "k", "v", "Wq", "bq", "Wk", "bk", "Wv", "bv",
                      "Wo", "bo", "E")}
    in_maps = _prep_inputs(**args)
    b2j.install_neuronx_cc_hook()

    partition_name = (nc.partition_id_tensor.name
                      if nc.partition_id_tensor else None)
    in_names, out_names, out_avals, zero_outs = [], [], [], []
    for alloc in nc.m.functions[0].allocations:
        if not isinstance(alloc, mb.MemoryLocationSet):
            continue
        name = alloc.memorylocations[0].name
        if alloc.kind == "ExternalInput":
            if name != partition_name:
                in_names.append(name)
        elif alloc.kind == "ExternalOutput":
            out_names.append(name)
            shape = tuple(alloc.tensor_shape)
            dtype = mb.dt.np(alloc.dtype)
            out_avals.append(jax.core.ShapedArray(shape, dtype))
            zero_outs.append(np.zeros(shape, dtype))
    n_params = len(in_names)
    n_outs = len(out_avals)
    all_names = in_names + out_names
    if partition_name is not None:
        all_names = all_names + [partition_name]

    def _body(*fargs):
        operands = list(fargs)
        if partition_name is not None:
            operands.append(b2j.partition_id_tensor())
        outs = b2j._bass_exec_p.bind(
            *operands, out_avals=tuple(out_avals), in_names=tuple(all_names),
            out_names=tuple(out_names), lowering_input_output_aliases=(),
            sim_require_finite=True, sim_require_nnan=True, nc=nc)
        return tuple(outs)

    devices = jax.devices()[:NCORES]
    mesh = Mesh(np.asarray(devices), ("core",))
    in_specs = (PartitionSpec("core"),) * (n_params + n_outs)
    out_specs = (PartitionSpec("core"),) * n_outs
    sharded = jax.jit(
        shard_map(_body, mesh=mesh, in_specs=in_specs, out_specs=out_specs,
                  check_rep=False),
        keep_unused=True)

    concat_in = [np.concatenate([np.asarray(in_maps[c][n])
                                 for c in range(NCORES)], axis=0)
                 for n in in_names]
    dev_in = [jax.device_put(a) for a in concat_in]
    concat_zero = [np.concatenate([z] * NCORES, axis=0) for z in zero_outs]

    dev_zero = [jax.device_put(z) for z in concat_zero]
    # warmup (compiles / caches)
    outs = sharded(*dev_in, *dev_zero)
    jax.block_until_ready(outs)

    t0 = _time.perf_counter()
    results = []
    for _ in range(iters):
        results.append(sharded(*dev_in, *dev_zero))
    jax.block_until_ready(results)
    t1 = _time.perf_counter()
    return (t1 - t0) / iters * 1e9


def kernel(**inputs):
    global _PROG, LAST_EXEC_NS
    args = {n: np.asarray(inputs[n], np.float32)
            for n in ("q", "k", "v", "Wq", "bq", "Wk", "bk", "Wv", "bv",
                      "Wo", "bo", "E")}
    mask = np.asarray(inputs["mask"], np.float32)

    causal = np.array_equal(mask, np.triu(np.ones((L, L), np.float32), k=1))
    if not causal:
        return _reference_numpy(mask=mask, **args)

    if _PROG is None:
        _PROG = build_program()
    in_maps = _prep_inputs(**args)
    trace = os.environ.get("KERNEL_TRACE", "0") == "1"
    try:
        res = run_bass_kernel_spmd(_PROG, in_maps, core_ids=list(range(NCORES)),
                                   trace=trace)
    except ModuleNotFoundError:
        # axon NTFF profiling hook unavailable in this container
        res = run_bass_kernel_spmd(_PROG, in_maps, core_ids=list(range(NCORES)),
                                   trace=False)
    LAST_EXEC_NS = res.exec_time_ns
    globals()["LAST_RESULTS"] = res

    full = np.zeros((B, L, DM), np.float32)
    for core in range(NCORES):
        full[core // HG] += res.results[core]["out"]
    return full



# revision 44
# speedup vs baseline: 66.8416x; 1.0546x over previous
"""Trainium2 Bass kernel for nn_MultiHeadAttention_21251498181338.

Music-Transformer-style MHA with relative position embeddings (Huang et al.
skew trick), B=2, L=2048, D=1024, H=16, causal mask.

Sharding: 8 cores = 2 batches x 4-head groups (tensor parallel per head).
Each core computes q/k/v projections for its 4 heads, causal attention with
relative position logits, and a partial output projection (Wo row-split).
Partials are summed on the host during unshard.

Device-side structure (per core):
  - Projections produce qh^T/kh^T in [head-depth on partitions] layout and
    vh in [keys on partitions] layout, so no transposes are needed anywhere
    except for the attention probabilities themselves.
  - P = exp(QK^T/8) * exp(Srel/8): the additive logit split is computed
    multiplicatively so the relative-position term can be skew-aligned
    independently of QK^T.
  - The skew is a single SBUF->SBUF DMA per (head, q-tile) using a flat
    access pattern with partition step (row_len - 1): row i is read with a
    column offset of -i, which is exactly the Huang et al. pad/reshape
    trick. Columns beyond the valid relative-index range are zeroed, which
    also implements the causal mask for free (P = Pqk * 0 = 0 there).
  - PV uses TensorE transposes of P tiles. The softmax denominators come
    for free from the fused multiply+reduce (tensor_tensor_reduce): the
    per-query row sums accumulate on the vector engine during the
    P = exp(QK)*exp(Srel) multiply, so no denominator matmuls are needed.
    The per-query reciprocals are turned into a partition-replicated
    [128, 128] tile via a DVE 32x32 stream-transpose plus two tiny
    SBUF->SBUF broadcast DMAs.
  - The two heads of each pair interleave their K=64 matmuls (different PE
    row-groups run concurrently) and share [128, P] psum tiles for PV via
    tile_position column halves, so both heads normalize in one op and
    land directly in the packed outT layout.
  - The attention output appears transposed [depth, queries], which is
    exactly the stationary-operand layout the output projection needs.
"""

import os
import sys

sys.path.insert(0, "/opt/trn_rl_repo")

import numpy as np
import ml_dtypes

import concourse.bass as bass
import concourse.mybir as mybir
import concourse.tile as tile
from concourse import bacc
from concourse.bass_utils import run_bass_kernel_spmd
from concourse.masks import make_identity

BF16 = mybir.dt.bfloat16
F32 = mybir.dt.float32
NPBF16 = ml_dtypes.bfloat16

USE_TTR = os.environ.get("K_TTR", "0") == "1"
USE_CHAIN = os.environ.get("K_CHAIN", "1") == "1"

B, L, DM, H, D = 2, 2048, 1024, 16, 64
HG = 4            # heads per core (head group)
NCORES = 8
P = 128
KT = DM // P      # 8 contraction tiles for projections
NIT = L // P      # 16 query tiles
SCALE = 1.0 / np.sqrt(D)  # 0.125

LAST_EXEC_NS = None

_PROG = None


def _ncj(it):
    # number of 512-wide key chunks for query tile `it` (causal)
    return it // 4 + 1


def build_program():
    nc = bacc.Bacc(
        "TRN2",
        target_bir_lowering=False,
        debug=False,
        enable_asserts=False,
        num_devices=NCORES,
    )

    # ---- External I/O ----
    xq = nc.dram_tensor("xq", [DM, L], BF16, kind="ExternalInput")  # q[b].T
    xk = nc.dram_tensor("xk", [DM, L], BF16, kind="ExternalInput")
    xv = nc.dram_tensor("xv", [DM, L], BF16, kind="ExternalInput")
    wq = nc.dram_tensor("wq", [DM, 2 * P], BF16, kind="ExternalInput")  # group cols
    wk = nc.dram_tensor("wk", [DM, 2 * P], BF16, kind="ExternalInput")
    wv = nc.dram_tensor("wv", [DM, 2 * P], BF16, kind="ExternalInput")
    wo = nc.dram_tensor("wo", [2, P, DM], BF16, kind="ExternalInput")  # [hp, 2h*64, dm]
    eT = nc.dram_tensor("eT", [2, P, L], BF16, kind="ExternalInput")   # [hp, 2h*64, r]
    bqk = nc.dram_tensor("bqk", [P, 6], F32, kind="ExternalInput")  # bq|bk|bv
    mask2_d = nc.dram_tensor("mask2", [2, P], BF16, kind="ExternalInput")
    bo_t = nc.dram_tensor("bo", [P, DM], F32, kind="ExternalInput")     # row-replicated
    out = nc.dram_tensor("out", [L, DM], F32, kind="ExternalOutput")

    with tile.TileContext(nc) as tc:
        with (
            tc.tile_pool(name="persist", bufs=1) as pp,
            tc.tile_pool(name="work", bufs=2) as wp,
            tc.tile_pool(name="small", bufs=4) as sp,
        ):
            # ---- persistent SBUF tensors ----
            ident = pp.tile([P, P], BF16)
            make_identity(nc, ident)

            wq_sb = pp.tile([P, KT, 2 * P], BF16)
            nc.sync.dma_start(wq_sb, wq.ap().rearrange("(t p) c -> p t c", p=P))
            wk_sb = pp.tile([P, KT, 2 * P], BF16)
            nc.sync.dma_start(wk_sb, wk.ap().rearrange("(t p) c -> p t c", p=P))
            wv_sb = pp.tile([P, KT, 2 * P], BF16)
            nc.sync.dma_start(wv_sb, wv.ap().rearrange("(t p) c -> p t c", p=P))
            wo_sb = pp.tile([P, 2, DM], BF16)
            nc.sync.dma_start(wo_sb, wo.ap().rearrange("h p m -> p h m"))
            eT_sb = pp.tile([P, 2, L], BF16)
            nc.sync.dma_start(eT_sb, eT.ap().rearrange("h p r -> p h r"))
            bqk_sb = pp.tile([P, 6], F32)
            nc.sync.dma_start(bqk_sb, bqk.ap())
            bo_sb = pp.tile([P, DM], F32)
            nc.sync.dma_start(bo_sb, bo_t.ap())

            qhT = pp.tile([P, 2, L], BF16)   # [64*hl+d, hp, i]
            khT = pp.tile([P, 2, L], BF16)
            vhT = pp.tile([P, 2, L], BF16)
            vh = pp.tile([P, NIT, HG, 66], BF16)  # [j in tile, jt, local head, d|1|pad]
            outT = pp.tile([P, 2, L], BF16)  # [64*hl+d, hp, i]

            # exp(Srel) band buffers: each use writes exp values into [0:W]
            # and memsets the 128-col skew overhang [W:W+128]; the diagonal
            # skew DMA never reads past W+127.
            XSE_W = 2176  # 2048 max band + 128 skew overhang
            xse_pp = [[pp.tile([P, XSE_W], BF16, name=f"xse{hl}_{par}")
                       for par in range(2)] for hl in range(2)]
            # head-half selector used to replicate each head's reciprocal
            # row across its 64 psum partitions (bf16: 0/1 exact).
            mask2 = pp.tile([2, P], BF16)
            nc.sync.dma_start(mask2, mask2_d.ap())
            if not (USE_TTR and USE_CHAIN):
                ones64 = pp.tile([P, 64], BF16)
                nc.gpsimd.memset(ones64, 1.0)

            # ---- Stage 1: projections ----
            # All three projections use the same dense N=512 streams with
            # the weight slice stationary, producing [head-depth on
            # partitions] outputs; vh's [keys on partitions] layout is then
            # recovered with 32 PE transposes.
            with (
                tc.tile_pool(name="xin", bufs=9) as xp,
                tc.tile_pool(name="ps1", bufs=4, space="PSUM") as ps1,
                tc.tile_pool(name="psvT", bufs=2, space="PSUM") as psvTp,
            ):
                stage1 = ((xv, wv_sb, vhT, 4), (xk, wk_sb, khT, 2),
                          (xq, wq_sb, qhT, 0))
                for src, wsb, dst, bcol in stage1:
                    # per-kt chunk DMAs: matmuls start as soon as the first
                    # chunk lands instead of waiting for the full tensor
                    chunks = []
                    for kt in range(KT):
                        ch = xp.tile([P, L], BF16, tag="xchunk")
                        nc.sync.dma_start(ch, src.ap()[kt * P:(kt + 1) * P, :])
                        chunks.append(ch)
                    # kt-outer per hp: one LDWEIGHTS feeds 4 back-to-back
                    # matmuls; 4 psum tiles accumulate in parallel
                    for hp in range(2):
                        pss = {}
                        for ic in range(L // 512):
                            pss[ic] = ps1.tile([P, 512], F32, tag="ps1",
                                               name=f"ps1_{ic}")
                        for kt in range(KT):
                            for ic in range(L // 512):
                                nc.tensor.matmul(
                                    pss[ic],
                                    wsb[:, kt, hp * P:(hp + 1) * P],
                                    chunks[kt][:, ic * 512:(ic + 1) * 512],
                                    start=(kt == 0),
                                    stop=(kt == KT - 1),
                                )
                        for ic in range(L // 512):
                            nc.vector.tensor_scalar_add(
                                dst[:, hp, ic * 512:(ic + 1) * 512],
                                pss[ic],
                                bqk_sb[:, bcol + hp:bcol + hp + 1],
                            )
                    if src is xk:
                        # vhT -> vh transposes, emitted here so the k-stream
                        # masks the wait on vhT's bias adds
                        for hp in range(2):
                            for jt in range(NIT):
                                pvt = psvTp.tile([P, P], BF16, tag="psvT")
                                nc.tensor.transpose(
                                    pvt, vhT[:, hp, jt * P:(jt + 1) * P],
                                    ident)
                                nc.vector.tensor_copy(
                                    vh[:, jt, 2 * hp:2 * hp + 2, 0:64],
                                    pvt.rearrange("p (l d) -> p l d", l=2),
                                )

            # ---- Stage 2: attention ----
            # The two heads of a pair are interleaved so adjacent K=64
            # matmuls target different PE row-groups and run concurrently.
            with (
                tc.tile_pool(name="psA", bufs=4 if (USE_TTR and USE_CHAIN)
                             else 3, space="PSUM") as psAp,
                tc.tile_pool(name="psT", bufs=2, space="PSUM") as psTp,
                tc.tile_pool(name="psO", bufs=2, space="PSUM") as psOp,
                tc.tile_pool(name="psD", bufs=1, space="PSUM") as psDp,
            ):
                # big and small q-tiles interleaved: every scheduling window
                # contains one dense matmul stream, so the PE stays busy
                # (and HAM stays unthrottled) while the small tiles' long
                # scalar/DMA chains drain.
                it_order = []
                for i in range(NIT // 2):
                    it_order += [NIT - 1 - i, i]
                itc = 0
                for it in it_order:
                    nkt = it + 1               # valid 128-key tiles
                    W = nkt * P                # valid band width (r cols)
                    ncj = (W + 511) // 512     # 512-wide chunks (last partial)
                    i0 = it * P
                    r_lo = L - P - i0          # first relative index in band
                    for hp in range(2):
                        par = itc & 1
                        itc += 1
                        q_stat = [qhT[64 * hl:64 * hl + 64, hp, i0:i0 + P]
                                  for hl in (0, 1)]
                        xse = [xse_pp[hl][par] for hl in (0, 1)]
                        for hl in (0, 1):
                            # zero the 128-col skew overhang; cols beyond
                            # W+127 are never read by the diagonal DMA
                            nc.gpsimd.memset(xse[hl][:, W:W + P], 0.0)

                        # exp(Srel/8) bands, [query on partitions, r on free]
                        for cs in range(ncj):
                            n = min(512, W - cs * 512)
                            for hl in (0, 1):
                                pb = 64 * hl
                                ps = psAp.tile([P, 512], F32, tag="psA")
                                nc.tensor.matmul(
                                    ps[:, :n],
                                    q_stat[hl],
                                    eT_sb[pb:pb + 64, hp,
                                          r_lo + cs * 512:r_lo + cs * 512 + n],
                                    start=True, stop=True,
                                )
                                nc.scalar.activation(
                                    xse[hl][:, cs * 512:cs * 512 + n], ps[:, :n],
                                    mybir.ActivationFunctionType.Exp, scale=SCALE,
                                )

                        # exp(QK^T/8), trimmed to the valid band
                        pqk = [wp.tile([P, 2048], BF16, tag=f"pqk{hl}", name=f"pqk{hl}")
                               for hl in (0, 1)]
                        for jc in range(ncj):
                            n = min(512, W - jc * 512)
                            for hl in (0, 1):
                                pb = 64 * hl
                                ps = psAp.tile([P, 512], F32, tag="psA")
                                nc.tensor.matmul(
                                    ps[:, :n],
                                    q_stat[hl],
                                    khT[pb:pb + 64, hp, jc * 512:jc * 512 + n],
                                    start=True, stop=True,
                                )
                                nc.scalar.activation(
                                    pqk[hl][:, jc * 512:jc * 512 + n], ps[:, :n],
                                    mybir.ActivationFunctionType.Exp, scale=SCALE,
                                )

                        # skew (SBUF->SBUF diagonal DMA), then the fused
                        # P = pqk * xsk multiply whose accumulator output IS
                        # the softmax denominator (per-query row sum).
                        rec_pack = (sp.tile([P, 2], F32, tag="recp")
                                    if USE_TTR and USE_CHAIN else None)
                        pm = []
                        for hl in (0, 1):
                            xsk = wp.tile([P, 2048], BF16, tag=f"xsk{hl}")
                            row_len = xse[hl].ap[0][0]
                            diag = bass.AP(
                                xse[hl].tensor, xse[hl].offset + 127,
                                [[row_len - 1, P], [1, W]],
                            )
                            nc.sync.dma_start(xsk[:, :W], diag)
                            pmt = wp.tile([P, 2048], BF16, tag=f"pm{hl}")
                            if USE_TTR:
                                nc.vector.tensor_tensor_reduce(
                                    out=pmt[:, :W],
                                    in0=pqk[hl][:, :W],
                                    in1=xsk[:, :W],
                                    scale=1.0,
                                    scalar=0.0,
                                    op0=mybir.AluOpType.mult,
                                    op1=mybir.AluOpType.add,
                                    accum_out=rec_pack[:, hl:hl + 1],
                                )
                            else:
                                nc.vector.tensor_tensor(
                                    pmt[:, :W], pqk[hl][:, :W], xsk[:, :W],
                                    mybir.AluOpType.mult,
                                )
                            pm.append(pmt)

                        # PV into column-half psum tiles: head hl occupies
                        # psum partitions [64*hl, 64*hl+64). NOTE: a psum
                        # bank tolerates only ONE open accumulation group --
                        # pso gets its bank to itself.
                        pso = psOp.tile([P, P], F32, tag="psO")
                        psd = (None if (USE_TTR and USE_CHAIN)
                               else psDp.tile([P, P], F32, tag="psD"))
                        for jb in range(ncj):
                            w_jb = min(512, W - jb * 512)
                            ntk = (w_jb + P - 1) // P
                            for hl in (0, 1):
                                pb = 64 * hl
                                lh = 2 * hp + hl
                                pst = psTp.tile([P, 512], BF16, tag="psT")
                                for t in range(ntk):
                                    nc.tensor.transpose(
                                        pst[:, t * P:(t + 1) * P],
                                        pm[hl][:, jb * 512 + t * P:
                                               jb * 512 + (t + 1) * P],
                                        ident,
                                    )
                                pts = sp.tile([P, 512], BF16, tag="pts")
                                nc.vector.tensor_copy(pts[:, :w_jb],
                                                      pst[:, :w_jb])
                                for t in range(ntk):
                                    jt = jb * 4 + t
                                    nc.tensor.matmul(
                                        pso[pb:pb + 64, :],
                                        vh[:, jt, lh, 0:64],
                                        pts[:, t * P:(t + 1) * P],
                                        start=(jt == 0),
                                        stop=(jt == nkt - 1),
                                        skip_group_check=True,
                                    )
                                    if psd is not None:
                                        nc.tensor.matmul(
                                            psd[pb:pb + 64, :],
                                            ones64,
                                            pts[:, t * P:(t + 1) * P],
                                            start=(jt == 0),
                                            stop=(jt == nkt - 1),
                                            skip_group_check=True,
                                        )

                        # per-query reciprocal -> partition-replicated psum:
                        # transpose rec [P,2] to a [2,128] psum row pair,
                        # copy to SBUF, then a mask2-stationary matmul
                        # replicates each head's row over its 64 partitions.
                        rrec = sp.tile([P, P], F32, tag="rrec")
                        if USE_TTR and USE_CHAIN:
                            rec_f = sp.tile([P, 2], F32, tag="recf")
                            nc.vector.reciprocal_approx_fast(
                                out=rec_f, in_=rec_pack)
                            rec_b = sp.tile([P, 2], BF16, tag="recb")
                            nc.vector.tensor_copy(rec_b, rec_f)
                            psrow = psTp.tile([2, P], BF16, tag="psT")
                            nc.tensor.transpose(psrow, rec_b, ident)
                            rrow = sp.tile([2, P], BF16, tag="rrow")
                            nc.vector.tensor_copy(rrow, psrow)
                            psrec = psTp.tile([P, P], F32, tag="psT")
                            nc.tensor.matmul(psrec, mask2, rrow,
                                             start=True, stop=True)
                            nc.vector.tensor_copy(rrec, psrec)
                        else:
                            nc.vector.reciprocal_approx_fast(
                                out=rrec, in_=psd)

                        # normalize both heads at once; writes land directly
                        # in the packed outT partition halves
                        nc.vector.tensor_tensor(
                            outT[:, hp, i0:i0 + P], pso, rrec,
                            mybir.AluOpType.mult,
                        )

            # ---- Stage 3: output projection (partial: this head group) ----
            with tc.tile_pool(name="ps3", bufs=2, space="PSUM") as ps3:
                for it in range(NIT):
                    pss3 = [ps3.tile([P, 512], F32, tag=f"ps3{mc}",
                                     name=f"ps3{mc}")
                            for mc in range(DM // 512)]
                    for hp in range(2):
                        for mc in range(DM // 512):
                            nc.tensor.matmul(
                                pss3[mc],
                                outT[:, hp, it * P:(it + 1) * P],
                                wo_sb[:, hp, mc * 512:(mc + 1) * 512],
                                start=(hp == 0),
                                stop=(hp == 1),
                            )
                    for mc in range(DM // 512):
                        osb = sp.tile([P, 512], F32, tag="osb")
                        nc.vector.tensor_tensor(
                            osb, pss3[mc], bo_sb[:, mc * 512:(mc + 1) * 512],
                            mybir.AluOpType.add,
                        )
                        nc.sync.dma_start(
                            out.ap()[it * P:(it + 1) * P, mc * 512:(mc + 1) * 512], osb
                        )
    nc.compile()
    return nc


def _prep_inputs(q, k, v, Wq, bq, Wk, bk, Wv, bv, Wo, bo, E):
    """Build the 8 per-core input maps (host-side shard + cast)."""
    in_maps = []
    for core in range(NCORES):
        b, g = core // HG, core % HG
        cols = slice(g * HG * D, (g + 1) * HG * D)  # this group's 256 cols
        # eT/wo packing: [hp, 64*hl + d, .]
        eTg = np.empty((2, P, L), NPBF16)
        wog = np.empty((2, P, DM), NPBF16)
        for hp in range(2):
            for hl in range(2):
                h = g * HG + 2 * hp + hl
                eTg[hp, 64 * hl:64 * hl + 64, :] = E[:, h * D:(h + 1) * D].T.astype(NPBF16)
                wog[hp, 64 * hl:64 * hl + 64, :] = Wo[h * D:(h + 1) * D, :].astype(NPBF16)
        bqk_a = np.empty((P, 6), np.float32)
        for hp in range(2):
            sl = slice(g * HG * D + hp * P, g * HG * D + (hp + 1) * P)
            bqk_a[:, hp] = bq[sl]
            bqk_a[:, 2 + hp] = bk[sl]
            bqk_a[:, 4 + hp] = bv[sl]
        bo_full = bo if g == 0 else np.zeros_like(bo)
        mask2_a = np.zeros((2, P), np.float32)
        mask2_a[0, 0:64] = 1.0
        mask2_a[1, 64:128] = 1.0
        in_maps.append({
            "mask2": mask2_a,
            "xq": np.ascontiguousarray(q[b].T).astype(NPBF16),
            "xk": np.ascontiguousarray(k[b].T).astype(NPBF16),
            "xv": np.ascontiguousarray(v[b].T).astype(NPBF16),
            "wq": np.ascontiguousarray(Wq[:, cols]).astype(NPBF16),
            "wk": np.ascontiguousarray(Wk[:, cols]).astype(NPBF16),
            "wv": np.ascontiguousarray(Wv[:, cols]).astype(NPBF16),
            "wo": wog,
            "eT": eTg,
            "bqk": bqk_a,
            "bo": np.ascontiguousarray(
                np.broadcast_to(bo_full[None, :], (P, DM))).astype(np.float32),
        })
    return in_maps


def _reference_numpy(q, k, v, mask, Wq, bq, Wk, bk, Wv, bv, Wo, bo, E):
    """Exact fallback for non-causal masks (never hit in practice)."""
    def split_heads(x):
        return np.moveaxis(x.reshape(*x.shape[:-1], H, D), -2, -3)
    qh = split_heads(q @ Wq + bq)
    kh = split_heads(k @ Wk + bk)
    vv = split_heads(v @ Wv + bv)
    eh = split_heads(E)
    QKt = np.einsum("bhqd,bhkd->bhqk", qh, kh)
    X = np.einsum("bhqd,hkd->bhqk", qh, eh)
    pad = np.pad(X, [(0, 0)] * 3 + [(1, 0)])
    s = pad.reshape(B, H, -1)[:, :, L:].reshape(B, H, L, L)
    logits = (QKt + s) / np.sqrt(D) + mask * -1e9
    m = logits.max(-1, keepdims=True)
    p = np.exp(logits - m)
    p /= p.sum(-1, keepdims=True)
    o = np.einsum("bhqk,bhkd->bhqd", p, vv)
    o = np.moveaxis(o, -3, -2).reshape(B, L, DM)
    return (o @ Wo + bo).astype(np.float32)


def benchmark(inputs, iters=20):
    """Amortized wall-clock of the sharded NEFF execution (device-resident
    inputs, back-to-back async dispatch). Returns est. ns per execution."""
    global _PROG
    import time as _time
    import jax
    from jax.sharding import Mesh, PartitionSpec
    from jax.experimental.shard_map import shard_map
    import concourse.bass2jax as b2j
    import concourse.mybir as mb

    if _PROG is None:
        _PROG = build_program()
    nc = _PROG
    args = {n: np.asarray(inputs[n], np.float32)
            for n in ("q", "k", "v", "Wq", "bq", "Wk", "bk", "Wv", "bv",
                      "Wo", "bo", "E")}
    in_maps = _prep_inputs(**args)
    b2j.install_neuronx_cc_hook()

    partition_name = (nc.partition_id_tensor.name
                      if nc.partition_id_tensor else None)
    in_names, out_names, out_avals, zero_outs = [], [], [], []
    for alloc in nc.m.functions[0].allocations:
        if not isinstance(alloc, mb.MemoryLocationSet):
            continue
        name = alloc.memorylocations[0].name
        if alloc.kind == "ExternalInput":
            if name != partition_name:
                in_names.append(name)
        elif alloc.kind == "ExternalOutput":
            out_names.append(name)
            shape = tuple(alloc.tensor_shape)
            dtype = mb.dt.np(alloc.dtype)
            out_avals.append(jax.core.ShapedArray(shape, dtype))
            zero_outs.append(np.zeros(shape, dtype))
    n_params = len(in_names)
    n_outs = len(out_avals)
    all_names = in_names + out_names
    if partition_name is not None:
        all_names = all_names + [partition_name]

    def _body(*fargs):
        operands = list(fargs)
        if partition_name is not None:
            operands.append(b2j.partition_id_tensor())
        outs = b2j._bass_exec_p.bind(
            *operands, out_avals=tuple(out_avals), in_names=tuple(all_names),
            out_names=tuple(out_names), lowering_input_output_aliases=(),
            sim_require_finite=True, sim_require_nnan=True, nc=nc)
        return tuple(outs)

    devices = jax.devices()[:NCORES]
    mesh = Mesh(np.asarray(devices), ("core",))
    in_specs = (PartitionSpec("core"),) * (n_params + n_outs)
    out_specs = (PartitionSpec("core"),) * n_outs
    sharded = jax.jit(
        shard_map(_body, mesh=mesh, in_specs=in_specs, out_specs=out_specs,
                  check_rep=False),
        keep_unused=True)

    concat_in = [np.concatenate([np.asarray(in_maps[c][n])
                                 for c in range(NCORES)], axis=0)
                 for n in in_names]
    dev_in = [jax.device_put(a) for a in concat_in]
    concat_zero = [np.concatenate([z] * NCORES, axis=0) for z in zero_outs]

    dev_zero = [jax.device_put(z) for z in concat_zero]
    # warmup (compiles / caches)
    outs = sharded(*dev_in, *dev_zero)
    jax.block_until_ready(outs)

    t0 = _time.perf_counter()
    results = []
    for _ in range(iters):
        results.append(sharded(*dev_in, *dev_zero))
    jax.block_until_ready(results)
    t1 = _time.perf_counter()
    return (t1 - t0) / iters * 1e9


def kernel(**inputs):
    global _PROG, LAST_EXEC_NS
    args = {n: np.asarray(inputs[n], np.float32)
            for n in ("q", "k", "v", "Wq", "bq", "Wk", "bk", "Wv", "bv",
                      "Wo", "bo", "E")}
    mask = np.asarray(inputs["mask"], np.float32)

    causal = np.array_equal(mask, np.triu(np.ones((L, L), np.float32), k=1))
    if not causal:
        return _reference_numpy(mask=mask, **args)

    if _PROG is None:
        _PROG = build_program()
    in_maps = _prep_inputs(**args)
    trace = os.environ.get("KERNEL_TRACE", "0") == "1"
    try:
        res = run_bass_kernel_spmd(_PROG, in_maps, core_ids=list(range(NCORES)),
                                   trace=trace)
    except ModuleNotFoundError:
        # axon NTFF profiling hook unavailable in this container
        res = run_bass_kernel_spmd(_PROG, in_maps, core_ids=list(range(NCORES)),
                                   trace=False)
    LAST_EXEC_NS = res.exec_time_ns
    globals()["LAST_RESULTS"] = res

    full = np.zeros((B, L, DM), np.float32)
    for core in range(NCORES):
        full[core // HG] += res.results[core]["out"]
    return full

